# revision 52
# baseline (speedup 1.0000x reference)
"""Trainium2 Bass kernel for nn_AttentionModel (B=4, S=4096, E=2048) on 8 cores.

Sharding: data-parallel over batch B (4) x tensor-parallel over the E output
dim of the Q projection (2). Core c handles batch b=c//2 and scores rows
e in [h*1024, (h+1)*1024) with h=c%2.

Algorithm (Gram reformulation — much less PE work than projecting Q/K/V):
  G = x^T x                      [E, E]   (symmetric: compute upper, mirror)
  A1T = G Wq'^T                  [E, EH]  (Wq' = Wq_half / sqrt(E))
  scoresT = Wk G Wq'^T + rank2   [E, EH]  = (Wk A1T) + bk u^T + rr bq'^T
  expT = exp(scoresT)            (softmax max-subtraction skipped: |scores|<~15)
  MhT = Wv expT                  [E, EH]  (unnormalized (attn Wv^T)^T)
  out = rsum * (MhT^T x^T + c')  [EH, S]  rsum/c' folded into PSUM eviction
where xsum = sum_s x[s,:] (host), u = Wq' xsum (host), rr = Wk xsum + S*bk
(host), c'[e] = sum_f expT[f,e] bv[f] and rsum[e] = 1/sum_f expT[f,e] (device,
via [ones|bv] K=128 matmuls + PE row->col transpose).

All GEMMs in float32r (full-rate fp32). Every matmul contracts over the
partition dim; moving chunks are 512 wide (1 PSUM bank, full rate).
"""

import sys

sys.path.insert(0, "/opt/trn_rl_repo")

from contextlib import ExitStack

import numpy as np

import concourse.bass as bass
import concourse.mybir as mybir
import concourse.tile as tile
from concourse import bacc
from concourse.bass_utils import run_bass_kernel_spmd
from concourse.masks import make_identity

f32 = mybir.dt.float32
f32r = mybir.dt.float32r

B, S, E = 4, 4096, 2048
EH = E // 2          # per-core scores rows (embed half)
NB = E // 128        # 16 embed blocks
SBK = S // 128       # 32 s k-tiles
PW = 512             # x panel width (G phase)
NP = E // PW         # 4 panels
N_CORES = 8


def build_kernel():
    nc = bacc.Bacc("TRN2", debug=False, target_bir_lowering=False)

    x_in = nc.dram_tensor("x", [S, E], f32r, kind="ExternalInput")
    xt = nc.dram_tensor("xt", [E, S], f32r, kind="ExternalInput")
    wqT = nc.dram_tensor("wqT", [E, EH], f32r, kind="ExternalInput")
    wkT = nc.dram_tensor("wkT", [E, E], f32r, kind="ExternalInput")
    wvT = nc.dram_tensor("wvT", [E, E], f32r, kind="ExternalInput")
    r2rep = nc.dram_tensor("r2rep", [128, 2, EH], f32, kind="ExternalInput")
    ovc = nc.dram_tensor("ovc", [128, NB, 4], f32r, kind="ExternalInput")
    outt = nc.dram_tensor("outt", [EH, S], f32, kind="ExternalOutput")

    with tile.TileContext(nc) as tc, ExitStack() as ctx:
        dram = ctx.enter_context(tc.tile_pool(name="dram", bufs=1, space="DRAM"))
        g_d = dram.tile([E, E], f32r)

        const = ctx.enter_context(tc.tile_pool(name="const", bufs=1))
        ident_f = const.tile([128, 128], f32)
        make_identity(nc, ident_f[:, :])
        ident = const.tile([128, 128], f32r)
        nc.vector.tensor_copy(ident[:, :], ident_f[:, :])
        ovc_sb = const.tile([128, NB, 4], f32r)
        nc.sync.dma_start(ovc_sb[:, :, :], ovc[:, :, :])
        sc822 = const.tile([128, 8, 2], f32)
        rsum = const.tile([128, 8], f32)
        cn = const.tile([128, 8], f32)

        # ---- Phase A: G = x^T x (upper strips + PE-transpose mirrors) ----
        # wq pool allocated below panels so its load can issue mid-phase-A.
        p_wq = tc.alloc_tile_pool(name="wq", bufs=1)
        with (
            tc.tile_pool(name="panels", bufs=1) as p_pan,
            tc.tile_pool(name="gsb", bufs=4, side="right") as p_gsb,
            tc.tile_pool(name="msb", bufs=2, side="right") as p_msb,
            tc.tile_pool(name="psA", bufs=1, space="PSUM") as p_psA,
            tc.tile_pool(name="pstA", bufs=2, space="PSUM") as p_pstA,
        ):
            slots = [None, None]

            def load_panel(pi, slot):
                # 4 separate subtile tiles -> per-subtile dependency
                # granularity (prefetch overlaps consumption of the old
                # panel; first matmuls only wait on subtile 0).
                subs = []
                for q in range(4):
                    t_q = p_pan.tile(
                        [128, 8, PW], f32r, tag=f"pan{slot}_{q}", name=f"pan{slot}_{q}"
                    )
                    subs.append(t_q)
                src = x_in[:, pi * PW:(pi + 1) * PW].rearrange(
                    "(sb p) c -> p sb c", p=128
                )
                for q in range(4):
                    eng = nc.sync if q % 2 == 0 else nc.scalar
                    eng.dma_start(subs[q][:, :, :], src[:, q * 8:(q + 1) * 8, :])
                slots[slot] = (pi, subs)

            def do_work(psup, s, mov_slot, stat_slot):
                pi_s, stat_subs = slots[stat_slot]
                pi_m, mov_subs = slots[mov_slot]
                assert pi_s == psup and pi_m == s
                pss = []
                for ii in range(4):
                    ps_ii = p_psA.tile([128, PW], f32, tag=f"ps{ii}", name=f"ps{ii}")
                    pss.append(ps_ii)
                for sbq in range(4):
                    for ii in range(4):
                        for sq in range(8):
                            nc.tensor.matmul(
                                pss[ii][:, :],
                                stat_subs[sbq][:, sq, ii * 128:(ii + 1) * 128],
                                mov_subs[sbq][:, sq, :],
                                start=(sbq == 0 and sq == 0),
                                stop=(sbq == 3 and sq == 7),
                            )
                for ii in range(4):
                    i = psup * 4 + ii
                    gsb = p_gsb.tile([128, PW], f32r, tag="gsb")
                    nc.scalar.copy(gsb[:, :], pss[ii][:, :])
                    nc.sync.dma_start(
                        g_d[i * 128:(i + 1) * 128, s * PW:(s + 1) * PW], gsb[:, :]
                    )
                    if s > psup:
                        pst = p_pstA.tile([128, PW], f32r, tag="pst")
                        for t4 in range(4):
                            nc.tensor.transpose(
                                pst[:, t4 * 128:(t4 + 1) * 128],
                                gsb[:, t4 * 128:(t4 + 1) * 128],
                                ident[:, :],
                            )
                        msb = p_msb.tile([128, PW], f32r, tag="msb")
                        nc.vector.tensor_copy(msb[:, :], pst[:, :])
                        nc.scalar.dma_start(
                            g_d[4 * s * 128:(4 * s + 4) * 128,
                                i * 128:(i + 1) * 128].rearrange(
                                "(q p) c -> p q c", p=128
                            ),
                            msb[:, :].rearrange("p (q c) -> p q c", c=128),
                        )

            load_panel(0, 0)
            load_panel(1, 1)
            do_work(0, 0, 0, 0)
            do_work(0, 1, 1, 0)
            load_panel(2, 1)
            do_work(0, 2, 1, 0)
            load_panel(3, 1)
            do_work(0, 3, 1, 0)
            load_panel(1, 0)
            do_work(1, 3, 1, 0)
            do_work(1, 1, 0, 0)
            load_panel(2, 1)
            do_work(1, 2, 1, 0)
            do_work(2, 2, 1, 1)
            load_panel(3, 0)
            # prefetch Wq'^T during the remaining ~100us of phase A compute
            wq_sb = p_wq.tile([128, NB, EH], f32r)
            wq_src = wqT[:, :].rearrange("(fb p) e -> p fb e", p=128)
            nc.sync.dma_start(wq_sb[:, 0:8, :], wq_src[:, 0:8, :])
            nc.scalar.dma_start(wq_sb[:, 8:16, :], wq_src[:, 8:16, :])
            do_work(2, 3, 0, 1)
            do_work(3, 3, 0, 0)

        # ---- Phase B: A1T = G Wq'^T  [E, EH] ----
        p_a1 = tc.alloc_tile_pool(name="a1t", bufs=1, side="right")
        a1t = p_a1.tile([128, NB, EH], f32r)
        with (
            tc.tile_pool(name="gcol", bufs=2, side="right") as p_gc,
            tc.tile_pool(name="psB", bufs=2, space="PSUM") as p_psB,
        ):
            for gb in range(NB):
                gcol = p_gc.tile([128, NB, 128], f32r, tag="gcol")
                nc.scalar.dma_start(
                    gcol[:, :, :],
                    g_d[:, gb * 128:(gb + 1) * 128].rearrange(
                        "(fb p) c -> p fb c", p=128
                    ),
                )
                ps = p_psB.tile([128, EH], f32, tag="ps")
                for fb in range(NB):
                    for ch in range(2):
                        nc.tensor.matmul(
                            ps[:, ch * 512:(ch + 1) * 512],
                            gcol[:, fb, :],
                            wq_sb[:, fb, ch * 512:(ch + 1) * 512],
                            start=(fb == 0),
                            stop=(fb == NB - 1),
                        )
                nc.vector.tensor_copy(a1t[:, gb, :], ps[:, :])
        p_wq.release()

        # ---- Phase C: expT = exp(Wk A1T + rank2), sums/c' via [1|bv] GEMM ----
        p_exp = tc.alloc_tile_pool(name="expt", bufs=1)
        expt = p_exp.tile([128, NB, EH], f32r)
        with (
            tc.tile_pool(name="r2", bufs=1) as p_r2,
            tc.tile_pool(name="wkcol", bufs=2) as p_wk,
            tc.tile_pool(name="psC", bufs=2, space="PSUM") as p_psC,
            tc.tile_pool(name="ps2", bufs=1, space="PSUM") as p_ps2,
            tc.tile_pool(name="pst2", bufs=1, space="PSUM") as p_pst2,
        ):
            r2rep_sb = p_r2.tile([128, 2, EH], f32)
            nc.sync.dma_start(r2rep_sb[:, :, :], r2rep[:, :, :])
            scs_rows = p_r2.tile([2, EH], f32r)
            ps2 = p_ps2.tile([2, EH], f32)

            def ovc_mm(fb):
                # row sums (ones) and c' (bv) in one K=128 series:
                # out rows [sum; c'] -- issued one fb behind the scores
                # loop so the PE never waits on the scalar-engine exp.
                for ch in range(2):
                    nc.tensor.matmul(
                        ps2[:, ch * 512:(ch + 1) * 512],
                        ovc_sb[:, fb, 0:2],
                        expt[:, fb, ch * 512:(ch + 1) * 512],
                        start=(fb == 0),
                        stop=(fb == NB - 1),
                    )

            for fb in range(NB):
                wkcol = p_wk.tile([128, NB, 128], f32r, tag="wkcol")
                nc.scalar.dma_start(
                    wkcol[:, :, :],
                    wkT[:, fb * 128:(fb + 1) * 128].rearrange(
                        "(gb p) c -> p gb c", p=128
                    ),
                )
                ps = p_psC.tile([128, EH], f32, tag="ps")
                for gb in range(NB):
                    for ch in range(2):
                        nc.tensor.matmul(
                            ps[:, ch * 512:(ch + 1) * 512],
                            wkcol[:, gb, :],
                            a1t[:, gb, ch * 512:(ch + 1) * 512],
                            start=(gb == 0),
                            stop=(gb == NB - 1),
                        )
                # rank-2 bias on the (idle) vector engine instead of the PE:
                # ps += bk[f]*u[e] ; ps += rr[f]*bq'[e]  (u/bq' host-replicated)
                nc.vector.scalar_tensor_tensor(
                    ps[:, :], r2rep_sb[:, 0, :], ovc_sb[:, fb, 2:3], ps[:, :],
                    mybir.AluOpType.mult, mybir.AluOpType.add,
                )
                nc.vector.scalar_tensor_tensor(
                    ps[:, :], r2rep_sb[:, 1, :], ovc_sb[:, fb, 3:4], ps[:, :],
                    mybir.AluOpType.mult, mybir.AluOpType.add,
                )
                nc.scalar.activation(
                    expt[:, fb, :], ps[:, :], mybir.ActivationFunctionType.Exp
                )
                if fb >= 1:
                    ovc_mm(fb - 1)
            ovc_mm(NB - 1)
            nc.vector.tensor_copy(scs_rows[:, :], ps2[:, :])
            pst2 = p_pst2.tile([128, 16], f32r)
            for eb in range(8):
                nc.tensor.transpose(
                    pst2[:, eb * 2:eb * 2 + 2],
                    scs_rows[:, eb * 128:(eb + 1) * 128],
                    ident[0:2, 0:2],
                )
            nc.vector.tensor_copy(
                sc822[:, :, :], pst2[:, :].rearrange("p (e t) -> p e t", t=2)
            )
            nc.vector.reciprocal(rsum[:, :], sc822[:, :, 0])
            nc.vector.tensor_tensor(
                cn[:, :], sc822[:, :, 1], rsum[:, :], mybir.AluOpType.mult
            )
        p_a1.release()

        # ---- Phase D: MhT = Wv expT  [E, EH] ----
        p_mh = tc.alloc_tile_pool(name="mht", bufs=1, side="right")
        mht = p_mh.tile([128, NB, EH], f32r)
        with (
            tc.tile_pool(name="wvcol", bufs=2) as p_wv,
            tc.tile_pool(name="psD", bufs=2, space="PSUM") as p_psD,
        ):
            for fpb in range(NB):
                wvcol = p_wv.tile([128, NB, 128], f32r, tag="wvcol")
                nc.sync.dma_start(
                    wvcol[:, :, :],
                    wvT[:, fpb * 128:(fpb + 1) * 128].rearrange(
                        "(fb p) c -> p fb c", p=128
                    ),
                )
                ps = p_psD.tile([128, EH], f32, tag="ps")
                for fb in range(NB):
                    for ch in range(2):
                        nc.tensor.matmul(
                            ps[:, ch * 512:(ch + 1) * 512],
                            wvcol[:, fb, :],
                            expt[:, fb, ch * 512:(ch + 1) * 512],
                            start=(fb == 0),
                            stop=(fb == NB - 1),
                        )
                nc.vector.tensor_copy(mht[:, fpb, :], ps[:, :])
        p_exp.release()

        # ---- Phase E: out = rsum * (MhT^T x^T + c') ----
        SC = 1024
        with (
            tc.tile_pool(name="xtc", bufs=2) as p_xt,
            tc.tile_pool(name="osb", bufs=2) as p_os,
            tc.tile_pool(name="psE", bufs=2, space="PSUM") as p_psE,
        ):
            for sck in range(S // SC):
                xtc = p_xt.tile([128, NB, SC], f32r, tag="xtc")
                src = xt[:, sck * SC:(sck + 1) * SC].rearrange(
                    "(fb p) s -> p fb s", p=128
                )
                # first chunk split across both queues (nothing else queued
                # yet); later chunks stay on sync so outt writes (scalar)
                # never delay the prefetch
                eng2 = nc.scalar if sck == 0 else nc.sync
                nc.sync.dma_start(xtc[:, 0:8, :], src[:, 0:8, :])
                eng2.dma_start(xtc[:, 8:16, :], src[:, 8:16, :])
                for eb in range(8):
                    ps = p_psE.tile([128, SC], f32, tag="ps")
                    for fpb in range(NB):
                        for ch in range(2):
                            nc.tensor.matmul(
                                ps[:, ch * 512:(ch + 1) * 512],
                                mht[:, fpb, eb * 128:(eb + 1) * 128],
                                xtc[:, fpb, ch * 512:(ch + 1) * 512],
                                start=(fpb == 0),
                                stop=(fpb == NB - 1),
                            )
                    osb = p_os.tile([128, SC], f32, tag="osb")
                    nc.vector.tensor_scalar(
                        osb[:, :], ps[:, :],
                        rsum[:, eb:eb + 1], cn[:, eb:eb + 1],
                        mybir.AluOpType.mult, mybir.AluOpType.add,
                    )
                    nc.scalar.dma_start(
                        outt[eb * 128:(eb + 1) * 128, sck * SC:(sck + 1) * SC],
                        osb[:, :],
                    )
        p_mh.release()

    nc.compile()
    return nc


_NC_CACHE = {}


def _get_nc():
    if "nc" not in _NC_CACHE:
        _NC_CACHE["nc"] = build_kernel()
    return _NC_CACHE["nc"]


def make_in_maps(x, Wq, bq, Wk, bk, Wv, bv):
    sc = np.float32(1.0 / np.sqrt(E))
    wkT = np.ascontiguousarray(Wk.T)
    # phase D stationary [f, g] must be Wv[f, g]: MhT = Wv^T expT
    wvT = np.ascontiguousarray(Wv)
    in_maps = []
    for c in range(N_CORES):
        b, h = c // 2, c % 2
        xb = np.ascontiguousarray(x[b])
        xsum = xb.sum(axis=0)
        wq_h = Wq[h * EH:(h + 1) * EH, :] * sc
        u = (wq_h @ xsum).astype(np.float32)
        rr = (Wk @ xsum + np.float32(S) * bk).astype(np.float32)
        bqp = (bq[h * EH:(h + 1) * EH] * sc).astype(np.float32)
        ovc = np.empty((128, NB, 4), np.float32)
        ovc[:, :, 0] = 1.0
        ovc[:, :, 1] = bv.reshape(NB, 128).T
        ovc[:, :, 2] = bk.reshape(NB, 128).T
        ovc[:, :, 3] = rr.reshape(NB, 128).T
        r2rep = np.empty((128, 2, EH), np.float32)
        r2rep[:, 0, :] = u[None, :]
        r2rep[:, 1, :] = bqp[None, :]
        in_maps.append({
            "x": xb,
            "xt": np.ascontiguousarray(xb.T),
            "wqT": np.ascontiguousarray(wq_h.T),
            "wkT": wkT,
            "wvT": wvT,
            "r2rep": r2rep,
            "ovc": ovc,
        })
    return in_maps


def run(in_maps, trace=False, **kwargs):
    nc = _get_nc()
    return run_bass_kernel_spmd(
        nc, in_maps, core_ids=list(range(N_CORES)), trace=trace, **kwargs
    )


def kernel(x, Wq, bq, Wk, bk, Wv, bv):
    x = np.asarray(x, dtype=np.float32)
    in_maps = make_in_maps(
        x,
        np.asarray(Wq, np.float32), np.asarray(bq, np.float32),
        np.asarray(Wk, np.float32), np.asarray(bk, np.float32),
        np.asarray(Wv, np.float32), np.asarray(bv, np.float32),
    )
    res = run(in_maps, trace=False)
    out = np.empty((B, E, S), dtype=np.float32)
    for c in range(N_CORES):
        b, h = c // 2, c % 2
        out[b, h * EH:(h + 1) * EH, :] = res.results[c]["outt"]
    return out


# revision 57
# speedup vs baseline: 1.0554x; 1.0554x over previous
"""Trainium2 Bass kernel for nn_AttentionModel (B=4, S=4096, E=2048) on 8 cores.

Sharding: data-parallel over batch B (4) x tensor-parallel over the E output
dim of the Q projection (2). Core c handles batch b=c//2 and scores rows
e in [h*1024, (h+1)*1024) with h=c%2.

Algorithm (Gram reformulation — much less PE work than projecting Q/K/V):
  G = x^T x                      [E, E]   (symmetric: compute upper, mirror)
  A1T = G Wq'^T                  [E, EH]  (Wq' = Wq_half / sqrt(E))
  scoresT = Wk G Wq'^T + rank2   [E, EH]  = (Wk A1T) + bk u^T + rr bq'^T
  expT = exp(scoresT)            (softmax max-subtraction skipped: |scores|<~15)
  MhT = Wv expT                  [E, EH]  (unnormalized (attn Wv^T)^T)
  out = rsum * (MhT^T x^T + c')  [EH, S]  rsum/c' folded into PSUM eviction
where xsum = sum_s x[s,:] (host), u = Wq' xsum (host), rr = Wk xsum + S*bk
(host), c'[e] = sum_f expT[f,e] bv[f] and rsum[e] = 1/sum_f expT[f,e] (device,
via [ones|bv] K=128 matmuls + PE row->col transpose).

All GEMMs in float32r (full-rate fp32). Every matmul contracts over the
partition dim; moving chunks are 512 wide (1 PSUM bank, full rate).
"""

import sys

sys.path.insert(0, "/opt/trn_rl_repo")

from contextlib import ExitStack

import numpy as np

import concourse.bass as bass
import concourse.mybir as mybir
import concourse.tile as tile
from concourse import bacc
from concourse.bass_utils import run_bass_kernel_spmd
from concourse.masks import make_identity

f32 = mybir.dt.float32
f32r = mybir.dt.float32r

B, S, E = 4, 4096, 2048
EH = E // 2          # per-core scores rows (embed half)
NB = E // 128        # 16 embed blocks
SBK = S // 128       # 32 s k-tiles
PW = 512             # x panel width (G phase)
NP = E // PW         # 4 panels
N_CORES = 8


def build_kernel():
    nc = bacc.Bacc("TRN2", debug=False, target_bir_lowering=False)

    x_in = nc.dram_tensor("x", [S, E], f32r, kind="ExternalInput")
    xt = nc.dram_tensor("xt", [E, S], f32r, kind="ExternalInput")
    wqT = nc.dram_tensor("wqT", [E, EH], f32r, kind="ExternalInput")
    wkT = nc.dram_tensor("wkT", [E, E], f32r, kind="ExternalInput")
    wvT = nc.dram_tensor("wvT", [E, E], f32r, kind="ExternalInput")
    r2rep = nc.dram_tensor("r2rep", [128, 2, EH], f32, kind="ExternalInput")
    ovc = nc.dram_tensor("ovc", [128, NB, 4], f32r, kind="ExternalInput")
    outt = nc.dram_tensor("outt", [EH, S], f32, kind="ExternalOutput")

    with tile.TileContext(nc) as tc, ExitStack() as ctx:
        dram = ctx.enter_context(tc.tile_pool(name="dram", bufs=1, space="DRAM"))
        g_d = dram.tile([E, E], f32r)

        const = ctx.enter_context(tc.tile_pool(name="const", bufs=1))
        ident_f = const.tile([128, 128], f32)
        make_identity(nc, ident_f[:, :])
        ident = const.tile([128, 128], f32r)
        nc.vector.tensor_copy(ident[:, :], ident_f[:, :])
        ovc_sb = const.tile([128, NB, 4], f32r)
        nc.sync.dma_start(ovc_sb[:, :, :], ovc[:, :, :])
        sc822 = const.tile([128, 8, 2], f32)
        rsum = const.tile([128, 8], f32)
        cn = const.tile([128, 8], f32)

        # ---- Phase A: G = x^T x (upper strips + PE-transpose mirrors) ----
        # wq pool allocated below panels so its load can issue mid-phase-A.
        p_wq = tc.alloc_tile_pool(name="wq", bufs=1)
        with (
            tc.tile_pool(name="panels", bufs=1) as p_pan,
            tc.tile_pool(name="gsb", bufs=4, side="right") as p_gsb,
            tc.tile_pool(name="msb", bufs=2, side="right") as p_msb,
            tc.tile_pool(name="psA", bufs=1, space="PSUM") as p_psA,
            tc.tile_pool(name="pstA", bufs=2, space="PSUM") as p_pstA,
        ):
            slots = [None, None]

            def load_panel(pi, slot):
                # 8 separate subtile tiles -> per-subtile dependency
                # granularity (prefetch overlaps consumption of the old
                # panel; first matmuls only wait on one 1MB subtile).
                subs = []
                for q in range(8):
                    t_q = p_pan.tile(
                        [128, 4, PW], f32r, tag=f"pan{slot}_{q}", name=f"pan{slot}_{q}"
                    )
                    subs.append(t_q)
                src = x_in[:, pi * PW:(pi + 1) * PW].rearrange(
                    "(sb p) c -> p sb c", p=128
                )
                for q in range(8):
                    eng = nc.sync if q % 2 == 0 else nc.scalar
                    eng.dma_start(subs[q][:, :, :], src[:, q * 4:(q + 1) * 4, :])
                slots[slot] = (pi, subs)

            def do_work(psup, s, mov_slot, stat_slot):
                pi_s, stat_subs = slots[stat_slot]
                pi_m, mov_subs = slots[mov_slot]
                assert pi_s == psup and pi_m == s
                pss = []
                for ii in range(4):
                    ps_ii = p_psA.tile([128, PW], f32, tag=f"ps{ii}", name=f"ps{ii}")
                    pss.append(ps_ii)
                for sbq in range(4):
                    for ii in range(4):
                        for sq in range(8):
                            sb = sbq * 8 + sq
                            nc.tensor.matmul(
                                pss[ii][:, :],
                                stat_subs[sb // 4][:, sb % 4,
                                                   ii * 128:(ii + 1) * 128],
                                mov_subs[sb // 4][:, sb % 4, :],
                                start=(sbq == 0 and sq == 0),
                                stop=(sbq == 3 and sq == 7),
                            )
                for ii in range(4):
                    i = psup * 4 + ii
                    gsb = p_gsb.tile([128, PW], f32r, tag="gsb")
                    nc.scalar.copy(gsb[:, :], pss[ii][:, :])
                    nc.sync.dma_start(
                        g_d[i * 128:(i + 1) * 128, s * PW:(s + 1) * PW], gsb[:, :]
                    )
                    if s > psup:
                        pst = p_pstA.tile([128, PW], f32r, tag="pst")
                        for t4 in range(4):
                            nc.tensor.transpose(
                                pst[:, t4 * 128:(t4 + 1) * 128],
                                gsb[:, t4 * 128:(t4 + 1) * 128],
                                ident[:, :],
                            )
                        msb = p_msb.tile([128, PW], f32r, tag="msb")
                        nc.vector.tensor_copy(msb[:, :], pst[:, :])
                        nc.scalar.dma_start(
                            g_d[4 * s * 128:(4 * s + 4) * 128,
                                i * 128:(i + 1) * 128].rearrange(
                                "(q p) c -> p q c", p=128
                            ),
                            msb[:, :].rearrange("p (q c) -> p q c", c=128),
                        )

            load_panel(0, 0)
            load_panel(1, 1)
            do_work(0, 0, 0, 0)
            do_work(0, 1, 1, 0)
            load_panel(2, 1)
            do_work(0, 2, 1, 0)
            load_panel(3, 1)
            do_work(0, 3, 1, 0)
            load_panel(1, 0)
            do_work(1, 3, 1, 0)
            do_work(1, 1, 0, 0)
            load_panel(2, 1)
            do_work(1, 2, 1, 0)
            do_work(2, 2, 1, 1)
            load_panel(3, 0)
            # prefetch Wq'^T during the remaining ~100us of phase A compute
            wq_sb = p_wq.tile([128, NB, EH], f32r)
            wq_src = wqT[:, :].rearrange("(fb p) e -> p fb e", p=128)
            nc.sync.dma_start(wq_sb[:, 0:8, :], wq_src[:, 0:8, :])
            nc.scalar.dma_start(wq_sb[:, 8:16, :], wq_src[:, 8:16, :])
            do_work(2, 3, 0, 1)
            do_work(3, 3, 0, 0)

        # ---- Phase B: A1T = G Wq'^T  [E, EH] ----
        # wkcol/r2 pools sit on the right, in fresh SBUF below B's gcol
        # ring, so phase C's first stationary strips + rank-2 rows prefetch
        # on the idle sync queue DURING phase B (no B->C boundary stall).
        p_a1 = tc.alloc_tile_pool(name="a1t", bufs=1, side="right")
        a1t = p_a1.tile([128, NB, EH], f32r)
        p_wk = tc.alloc_tile_pool(name="wkcol", bufs=2, side="right")
        p_r2 = tc.alloc_tile_pool(name="r2", bufs=1, side="right")
        r2rep_sb = p_r2.tile([128, 2, EH], f32)
        scs_rows = p_r2.tile([2, EH], f32r)
        wk_pre = []
        for pf in range(2):
            wk_t = p_wk.tile([128, NB, 128], f32r, tag="wkcol", name=f"wkpre{pf}")
            wk_pre.append(wk_t)
        with (
            tc.tile_pool(name="gcol", bufs=2, side="right") as p_gc,
            tc.tile_pool(name="psB", bufs=2, space="PSUM") as p_psB,
        ):
            for gb in range(NB):
                gcol = p_gc.tile([128, NB, 128], f32r, tag="gcol")
                nc.scalar.dma_start(
                    gcol[:, :, :],
                    g_d[:, gb * 128:(gb + 1) * 128].rearrange(
                        "(fb p) c -> p fb c", p=128
                    ),
                )
                if gb == 1:
                    nc.sync.dma_start(r2rep_sb[:, :, :], r2rep[:, :, :])
                if gb in (2, 3):
                    pf = gb - 2
                    nc.sync.dma_start(
                        wk_pre[pf][:, :, :],
                        wkT[:, pf * 128:(pf + 1) * 128].rearrange(
                            "(gb2 p) c -> p gb2 c", p=128
                        ),
                    )
                ps = p_psB.tile([128, EH], f32, tag="ps")
                for fb in range(NB):
                    for ch in range(2):
                        nc.tensor.matmul(
                            ps[:, ch * 512:(ch + 1) * 512],
                            gcol[:, fb, :],
                            wq_sb[:, fb, ch * 512:(ch + 1) * 512],
                            start=(fb == 0),
                            stop=(fb == NB - 1),
                        )
                nc.vector.tensor_copy(a1t[:, gb, :], ps[:, :])
        p_wq.release()

        # ---- Phase C: expT = exp(Wk A1T + rank2), sums/c' via [1|bv] GEMM ----
        p_exp = tc.alloc_tile_pool(name="expt", bufs=1)
        expt = p_exp.tile([128, NB, EH], f32r)
        with (
            tc.tile_pool(name="psC", bufs=2, space="PSUM") as p_psC,
            tc.tile_pool(name="ps2", bufs=1, space="PSUM") as p_ps2,
            tc.tile_pool(name="pst2", bufs=1, space="PSUM") as p_pst2,
        ):
            ps2 = p_ps2.tile([2, EH], f32)

            def ovc_mm(fb):
                # row sums (ones) and c' (bv) in one K=128 series:
                # out rows [sum; c'] -- issued one fb behind the scores
                # loop so the PE never waits on the scalar-engine exp.
                for ch in range(2):
                    nc.tensor.matmul(
                        ps2[:, ch * 512:(ch + 1) * 512],
                        ovc_sb[:, fb, 0:2],
                        expt[:, fb, ch * 512:(ch + 1) * 512],
                        start=(fb == 0),
                        stop=(fb == NB - 1),
                    )

            for fb in range(NB):
                if fb < 2:
                    wkcol = wk_pre[fb]
                else:
                    wkcol = p_wk.tile([128, NB, 128], f32r, tag="wkcol")
                    nc.scalar.dma_start(
                        wkcol[:, :, :],
                        wkT[:, fb * 128:(fb + 1) * 128].rearrange(
                            "(gb p) c -> p gb c", p=128
                        ),
                    )
                ps = p_psC.tile([128, EH], f32, tag="ps")
                for gb in range(NB):
                    for ch in range(2):
                        nc.tensor.matmul(
                            ps[:, ch * 512:(ch + 1) * 512],
                            wkcol[:, gb, :],
                            a1t[:, gb, ch * 512:(ch + 1) * 512],
                            start=(gb == 0),
                            stop=(gb == NB - 1),
                        )
                # rank-2 bias on the (idle) vector engine instead of the PE:
                # ps += bk[f]*u[e] ; ps += rr[f]*bq'[e]  (u/bq' host-replicated)
                nc.vector.scalar_tensor_tensor(
                    ps[:, :], r2rep_sb[:, 0, :], ovc_sb[:, fb, 2:3], ps[:, :],
                    mybir.AluOpType.mult, mybir.AluOpType.add,
                )
                nc.vector.scalar_tensor_tensor(
                    ps[:, :], r2rep_sb[:, 1, :], ovc_sb[:, fb, 3:4], ps[:, :],
                    mybir.AluOpType.mult, mybir.AluOpType.add,
                )
                nc.scalar.activation(
                    expt[:, fb, :], ps[:, :], mybir.ActivationFunctionType.Exp
                )
                if fb >= 1:
                    ovc_mm(fb - 1)
            ovc_mm(NB - 1)
            nc.vector.tensor_copy(scs_rows[:, :], ps2[:, :])
            pst2 = p_pst2.tile([128, 16], f32r)
            for eb in range(8):
                nc.tensor.transpose(
                    pst2[:, eb * 2:eb * 2 + 2],
                    scs_rows[:, eb * 128:(eb + 1) * 128],
                    ident[0:2, 0:2],
                )
            nc.vector.tensor_copy(
                sc822[:, :, :], pst2[:, :].rearrange("p (e t) -> p e t", t=2)
            )
            nc.vector.reciprocal(rsum[:, :], sc822[:, :, 0])
            nc.vector.tensor_tensor(
                cn[:, :], sc822[:, :, 1], rsum[:, :], mybir.AluOpType.mult
            )
        p_r2.release()
        p_wk.release()
        p_a1.release()

        # ---- Phase D: MhT = Wv expT  [E, EH] ----
        p_mh = tc.alloc_tile_pool(name="mht", bufs=1, side="right")
        mht = p_mh.tile([128, NB, EH], f32r)
        with (
            tc.tile_pool(name="wvcol", bufs=2) as p_wv,
            tc.tile_pool(name="psD", bufs=2, space="PSUM") as p_psD,
        ):
            for fpb in range(NB):
                wvcol = p_wv.tile([128, NB, 128], f32r, tag="wvcol")
                nc.sync.dma_start(
                    wvcol[:, :, :],
                    wvT[:, fpb * 128:(fpb + 1) * 128].rearrange(
                        "(fb p) c -> p fb c", p=128
                    ),
                )
                ps = p_psD.tile([128, EH], f32, tag="ps")
                for fb in range(NB):
                    for ch in range(2):
                        nc.tensor.matmul(
                            ps[:, ch * 512:(ch + 1) * 512],
                            wvcol[:, fb, :],
                            expt[:, fb, ch * 512:(ch + 1) * 512],
                            start=(fb == 0),
                            stop=(fb == NB - 1),
                        )
                nc.vector.tensor_copy(mht[:, fpb, :], ps[:, :])
        p_exp.release()

        # ---- Phase E: out = rsum * (MhT^T x^T + c') ----
        SC = 1024
        with (
            tc.tile_pool(name="xtc", bufs=2) as p_xt,
            tc.tile_pool(name="osb", bufs=2) as p_os,
            tc.tile_pool(name="psE", bufs=2, space="PSUM") as p_psE,
        ):
            for sck in range(S // SC):
                xtc = p_xt.tile([128, NB, SC], f32r, tag="xtc")
                src = xt[:, sck * SC:(sck + 1) * SC].rearrange(
                    "(fb p) s -> p fb s", p=128
                )
                # first chunk split across both queues (nothing else queued
                # yet); later chunks stay on sync so outt writes (scalar)
                # never delay the prefetch
                eng2 = nc.scalar if sck == 0 else nc.sync
                nc.sync.dma_start(xtc[:, 0:8, :], src[:, 0:8, :])
                eng2.dma_start(xtc[:, 8:16, :], src[:, 8:16, :])
                for eb in range(8):
                    ps = p_psE.tile([128, SC], f32, tag="ps")
                    for fpb in range(NB):
                        for ch in range(2):
                            nc.tensor.matmul(
                                ps[:, ch * 512:(ch + 1) * 512],
                                mht[:, fpb, eb * 128:(eb + 1) * 128],
                                xtc[:, fpb, ch * 512:(ch + 1) * 512],
                                start=(fpb == 0),
                                stop=(fpb == NB - 1),
                            )
                    osb = p_os.tile([128, SC], f32, tag="osb")
                    nc.vector.tensor_scalar(
                        osb[:, :], ps[:, :],
                        rsum[:, eb:eb + 1], cn[:, eb:eb + 1],
                        mybir.AluOpType.mult, mybir.AluOpType.add,
                    )
                    nc.scalar.dma_start(
                        outt[eb * 128:(eb + 1) * 128, sck * SC:(sck + 1) * SC],
                        osb[:, :],
                    )
        p_mh.release()

    nc.compile()
    return nc


_NC_CACHE = {}


def _get_nc():
    if "nc" not in _NC_CACHE:
        _NC_CACHE["nc"] = build_kernel()
    return _NC_CACHE["nc"]


def make_in_maps(x, Wq, bq, Wk, bk, Wv, bv):
    sc = np.float32(1.0 / np.sqrt(E))
    wkT = np.ascontiguousarray(Wk.T)
    # phase D stationary [f, g] must be Wv[f, g]: MhT = Wv^T expT
    wvT = np.ascontiguousarray(Wv)
    in_maps = []
    for c in range(N_CORES):
        b, h = c // 2, c % 2
        xb = np.ascontiguousarray(x[b])
        xsum = xb.sum(axis=0)
        wq_h = Wq[h * EH:(h + 1) * EH, :] * sc
        u = (wq_h @ xsum).astype(np.float32)
        rr = (Wk @ xsum + np.float32(S) * bk).astype(np.float32)
        bqp = (bq[h * EH:(h + 1) * EH] * sc).astype(np.float32)
        ovc = np.empty((128, NB, 4), np.float32)
        ovc[:, :, 0] = 1.0
        ovc[:, :, 1] = bv.reshape(NB, 128).T
        ovc[:, :, 2] = bk.reshape(NB, 128).T
        ovc[:, :, 3] = rr.reshape(NB, 128).T
        r2rep = np.empty((128, 2, EH), np.float32)
        r2rep[:, 0, :] = u[None, :]
        r2rep[:, 1, :] = bqp[None, :]
        in_maps.append({
            "x": xb,
            "xt": np.ascontiguousarray(xb.T),
            "wqT": np.ascontiguousarray(wq_h.T),
            "wkT": wkT,
            "wvT": wvT,
            "r2rep": r2rep,
            "ovc": ovc,
        })
    return in_maps


def run(in_maps, trace=False, **kwargs):
    nc = _get_nc()
    return run_bass_kernel_spmd(
        nc, in_maps, core_ids=list(range(N_CORES)), trace=trace, **kwargs
    )


def kernel(x, Wq, bq, Wk, bk, Wv, bv):
    x = np.asarray(x, dtype=np.float32)
    in_maps = make_in_maps(
        x,
        np.asarray(Wq, np.float32), np.asarray(bq, np.float32),
        np.asarray(Wk, np.float32), np.asarray(bk, np.float32),
        np.asarray(Wv, np.float32), np.asarray(bv, np.float32),
    )
    res = run(in_maps, trace=False)
    out = np.empty((B, E, S), dtype=np.float32)
    for c in range(N_CORES):
        b, h = c // 2, c % 2
        out[b, h * EH:(h + 1) * EH, :] = res.results[c]["outt"]
    return out
